# revision 2
# baseline (speedup 1.0000x reference)
"""Grouped-query attention with sliding-window mask on 8 Trainium2 cores.

Sharding: core c handles (batch b = c // 4, kv-head group hk = c % 4).
Each core projects q (4 query heads) / k / v for its group, applies RoPE,
runs windowed attention, and computes a partial output projection
out_partial = attn_heads @ Wo[hk block].  The host sums the 4 partials
per batch.

Problem constants are hardcoded (B=2, N=2048, E=2048, H=16, G=4, D=128,
WIN=256); inputs arrive as full unsharded arrays.
"""

import sys

for _p in ("/opt/trn_rl_repo", "/opt/pypackages"):
    if _p not in sys.path:
        sys.path.insert(0, _p)

from contextlib import ExitStack

import numpy as np

import concourse.bacc as bacc
import concourse.mybir as mybir
import concourse.tile as tile
from concourse.bass_utils import run_bass_kernel_spmd

B, N, E = 2, 2048, 2048
H, G, WIN = 16, 4, 256
HK = H // G          # 4 kv heads
D = E // H           # 128
SCALE = D ** -0.5
NCORES = 8
P = 128
NT = N // P          # 16 n-tiles
EC = E // P          # 16 contraction chunks
HALF = N // 2
F32 = mybir.dt.float32
MASK_VAL = -1.0e30

_compiled = {}


def _rope(nc, rtmp, dst, src, c_ap, s_ap, nblocks):
    """RoPE on [128, nblocks*128] (pairs adjacent along free dim).

    dst[2t]   = src[2t]*cos[t] - src[2t+1]*sin[t]
    dst[2t+1] = src[2t]*sin[t] + src[2t+1]*cos[t]
    """
    for blk in range(nblocks):
        b0 = blk * P
        x0 = src[:, b0 + 0:b0 + P:2]
        x1 = src[:, b0 + 1:b0 + P:2]
        t0 = rtmp.tile([P, D // 2], F32, tag="t0")
        t1 = rtmp.tile([P, D // 2], F32, tag="t1")
        nc.vector.tensor_mul(t0[:], x0, c_ap)
        nc.vector.tensor_mul(t1[:], x1, s_ap)
        nc.vector.tensor_sub(dst[:, b0 + 0:b0 + P:2], t0[:], t1[:])
        t2 = rtmp.tile([P, D // 2], F32, tag="t2")
        t3 = rtmp.tile([P, D // 2], F32, tag="t3")
        nc.vector.tensor_mul(t2[:], x0, s_ap)
        nc.vector.tensor_mul(t3[:], x1, c_ap)
        nc.vector.tensor_add(dst[:, b0 + 1:b0 + P:2], t2[:], t3[:])


def _build():
    nc = bacc.Bacc("TRN2", target_bir_lowering=False, debug=False)

    xt_d = nc.dram_tensor("xt", [E, N], F32, kind="ExternalInput")
    wq_d = nc.dram_tensor("wq", [E, G * D], F32, kind="ExternalInput")
    wkv_d = nc.dram_tensor("wkv", [E, 2 * D], F32, kind="ExternalInput")
    wo_d = nc.dram_tensor("wo", [G * D, E], F32, kind="ExternalInput")
    cos_d = nc.dram_tensor("coss", [N, D // 2], F32, kind="ExternalInput")
    sin_d = nc.dram_tensor("sins", [N, D // 2], F32, kind="ExternalInput")
    mask_d = nc.dram_tensor("mask", [P, 3 * P], F32, kind="ExternalInput")
    eye_d = nc.dram_tensor("eye", [P, P], F32, kind="ExternalInput")
    out_d = nc.dram_tensor("out", [N, E], F32, kind="ExternalOutput")

    xt3 = xt_d.ap().rearrange("(c p) n -> c p n", p=P)
    wq3 = wq_d.ap().rearrange("(c p) m -> c p m", p=P)
    wkv3 = wkv_d.ap().rearrange("(c p) m -> c p m", p=P)
    wo3 = wo_d.ap().rearrange("(g p) e -> g p e", p=P)

    with tile.TileContext(nc) as tc, ExitStack() as top:
        pers = top.enter_context(tc.tile_pool(name="pers", bufs=1))
        qt_sb = pers.tile([P, G * N], F32, tag="qt")      # per head g: [:, g*N + n]
        kt_sb = pers.tile([P, N], F32, tag="kt")          # [d, n]
        v_sb = pers.tile([P, N], F32, tag="v")            # tile t: [:, t*128] = v[t*128+p, d]
        cos_sb = pers.tile([P, NT * (D // 2)], F32, tag="cos")
        sin_sb = pers.tile([P, NT * (D // 2)], F32, tag="sin")
        mask_sb = pers.tile([P, 3 * P], F32, tag="mask")
        eye_sb = pers.tile([P, P], F32, tag="eye")

        nc.sync.dma_start(
            cos_sb[:].rearrange("p (t d) -> p t d", d=D // 2),
            cos_d.ap().rearrange("(t p) d -> p t d", p=P),
        )
        nc.sync.dma_start(
            sin_sb[:].rearrange("p (t d) -> p t d", d=D // 2),
            sin_d.ap().rearrange("(t p) d -> p t d", p=P),
        )
        nc.sync.dma_start(mask_sb[:], mask_d.ap())
        nc.sync.dma_start(eye_sb[:], eye_d.ap())

        # ---------------- Phase A: projections + RoPE + transposes ---------
        with ExitStack() as pha:
            wq_pool = pha.enter_context(tc.tile_pool(name="wq", bufs=EC))
            wkv_pool = pha.enter_context(tc.tile_pool(name="wkv", bufs=EC))
            xt_pool = pha.enter_context(tc.tile_pool(name="xtp", bufs=EC))
            qrot_pool = pha.enter_context(tc.tile_pool(name="qrot", bufs=2))
            krot_pool = pha.enter_context(tc.tile_pool(name="krot", bufs=2))
            rtmp = pha.enter_context(tc.tile_pool(name="rtmp", bufs=4))
            qps_pool = pha.enter_context(
                tc.tile_pool(name="qps", bufs=2, space="PSUM"))
            kvps_pool = pha.enter_context(
                tc.tile_pool(name="kvps", bufs=2, space="PSUM"))
            trps_pool = pha.enter_context(
                tc.tile_pool(name="trps", bufs=2, space="PSUM"))

            wqs = [wq_pool.tile([P, G * D], F32, tag="wq", name="wqt") for _ in range(EC)]
            wkvs = [wkv_pool.tile([P, 2 * D], F32, tag="wkv", name="wkvt") for _ in range(EC)]
            for e in range(EC):
                nc.sync.dma_start(wqs[e][:], wq3[e])
                nc.sync.dma_start(wkvs[e][:], wkv3[e])

            for h in range(2):
                xts = [xt_pool.tile([P, HALF], F32, tag="xt", name="xtt") for _ in range(EC)]
                for e in range(EC):
                    nc.sync.dma_start(xts[e][:], xt3[e][:, h * HALF:(h + 1) * HALF])

                for t in range(NT // 2):
                    T = h * (NT // 2) + t
                    q_ps = qps_pool.tile([P, G * D], F32, tag="qps")
                    kv_ps = kvps_pool.tile([P, 2 * D], F32, tag="kvps")
                    for e in range(EC):
                        nc.tensor.matmul(
                            q_ps[:], xts[e][:, t * P:(t + 1) * P], wqs[e][:],
                            start=(e == 0), stop=(e == EC - 1))
                    for e in range(EC):
                        nc.tensor.matmul(
                            kv_ps[:], xts[e][:, t * P:(t + 1) * P], wkvs[e][:],
                            start=(e == 0), stop=(e == EC - 1))

                    c_ap = cos_sb[:, T * (D // 2):(T + 1) * (D // 2)]
                    s_ap = sin_sb[:, T * (D // 2):(T + 1) * (D // 2)]
                    q_rot = qrot_pool.tile([P, G * D], F32, tag="qrot")
                    k_rot = krot_pool.tile([P, D], F32, tag="krot")
                    _rope(nc, rtmp, q_rot, q_ps, c_ap, s_ap, G)
                    _rope(nc, rtmp, k_rot, kv_ps, c_ap, s_ap, 1)
                    nc.scalar.copy(v_sb[:, T * P:(T + 1) * P], kv_ps[:, D:2 * D])

                    for g in range(G):
                        tq = trps_pool.tile([P, P], F32, tag="trq")
                        nc.tensor.transpose(
                            tq[:], q_rot[:, g * P:(g + 1) * P], eye_sb[:])
                        nc.scalar.copy(
                            qt_sb[:, g * N + T * P: g * N + (T + 1) * P], tq[:])
                    tk = trps_pool.tile([P, P], F32, tag="trq")
                    nc.tensor.transpose(tk[:], k_rot[:], eye_sb[:])
                    nc.scalar.copy(kt_sb[:, T * P:(T + 1) * P], tk[:])

        # ---------------- Phase B: attention + output projection -----------
        with ExitStack() as phb:
            wo_pool = phb.enter_context(tc.tile_pool(name="wo", bufs=G))
            sm_pool = phb.enter_context(tc.tile_pool(name="sm", bufs=3))
            stat_pool = phb.enter_context(tc.tile_pool(name="stat", bufs=6))
            pt_pool = phb.enter_context(tc.tile_pool(name="pt", bufs=2))
            ao_pool = phb.enter_context(tc.tile_pool(name="ao", bufs=2))
            osb_pool = phb.enter_context(tc.tile_pool(name="osb", bufs=2))
            sps_pool = phb.enter_context(
                tc.tile_pool(name="sps", bufs=2, space="PSUM"))
            ptps_pool = phb.enter_context(
                tc.tile_pool(name="ptps", bufs=1, space="PSUM"))
            aops_pool = phb.enter_context(
                tc.tile_pool(name="aops", bufs=1, space="PSUM"))
            wops_pool = phb.enter_context(
                tc.tile_pool(name="wops", bufs=1, space="PSUM"))

            wos = [wo_pool.tile([P, E], F32, tag="wo", name="wot") for _ in range(G)]
            for g in range(G):
                nc.sync.dma_start(wos[g][:], wo3[g])

            for qt in range(NT):
                w0 = max(0, qt * P - WIN)
                wN = (qt + 1) * P - w0          # 128 / 256 / 384
                nk = wN // P
                ao_sb = ao_pool.tile([P, G * P], F32, tag="ao")
                for g in range(G):
                    s_ps = sps_pool.tile([P, 3 * P], F32, tag="sps")
                    nc.tensor.matmul(
                        s_ps[:, :wN],
                        qt_sb[:, g * N + qt * P: g * N + (qt + 1) * P],
                        kt_sb[:, w0:w0 + wN],
                        start=True, stop=True)
                    s_sb = sm_pool.tile([P, 3 * P], F32, tag="ssb")
                    nc.vector.tensor_add(
                        s_sb[:, :wN], s_ps[:, :wN], mask_sb[:, 3 * P - wN:])
                    negm = stat_pool.tile([P, 1], F32, tag="negm")
                    nc.vector.reduce_max(
                        negm[:], s_sb[:, :wN], axis=mybir.AxisListType.X,
                        negate=True)
                    p_sb = sm_pool.tile([P, 3 * P], F32, tag="psb")
                    denom = stat_pool.tile([P, 1], F32, tag="den")
                    nc.scalar.activation(
                        p_sb[:, :wN], s_sb[:, :wN],
                        mybir.ActivationFunctionType.Exp,
                        bias=negm[:], scale=1.0, accum_out=denom[:])
                    recip = stat_pool.tile([P, 1], F32, tag="rec")
                    nc.vector.reciprocal(recip[:], denom[:])
                    nc.vector.tensor_scalar_mul(p_sb[:, :wN], p_sb[:, :wN], recip[:])

                    pt_ps = ptps_pool.tile([P, 3 * P], F32, tag="ptps")
                    for j in range(nk):
                        nc.tensor.transpose(
                            pt_ps[:, j * P:(j + 1) * P],
                            p_sb[:, j * P:(j + 1) * P], eye_sb[:])
                    pt_sb = pt_pool.tile([P, 3 * P], F32, tag="ptsb")
                    nc.scalar.copy(pt_sb[:, :wN], pt_ps[:, :wN])

                    ao_ps = aops_pool.tile([P, P], F32, tag="aops")
                    for j in range(nk):
                        kb = w0 // P + j
                        nc.tensor.matmul(
                            ao_ps[:],
                            v_sb[:, kb * P:(kb + 1) * P],
                            pt_sb[:, j * P:(j + 1) * P],
                            start=(j == 0), stop=(j == nk - 1))
                    nc.scalar.copy(ao_sb[:, g * P:(g + 1) * P], ao_ps[:])

                wo_ps = wops_pool.tile([P, E], F32, tag="wops")
                for eb in range(4):
                    for g in range(G):
                        nc.tensor.matmul(
                            wo_ps[:, eb * 512:(eb + 1) * 512],
                            ao_sb[:, g * P:(g + 1) * P],
                            wos[g][:, eb * 512:(eb + 1) * 512],
                            start=(g == 0), stop=(g == G - 1))
                out_sb = osb_pool.tile([P, E], F32, tag="outsb")
                for eb in range(4):
                    nc.scalar.copy(
                        out_sb[:, eb * 512:(eb + 1) * 512],
                        wo_ps[:, eb * 512:(eb + 1) * 512])
                nc.sync.dma_start(out_d.ap()[qt * P:(qt + 1) * P, :], out_sb[:])

    nc.compile()
    return nc


def _host_inputs(x, rope_cos, rope_sin, Wq, Wk, Wv, Wo):
    """Build the 8 per-core input maps."""
    band = np.full((P, 3 * P), MASK_VAL, dtype=np.float32)
    r = np.arange(P)[:, None]
    c = np.arange(3 * P)[None, :]
    band[(c > r) & (c <= r + WIN)] = 0.0
    eye = np.eye(P, dtype=np.float32)

    in_maps = []
    for core in range(NCORES):
        b, hk = divmod(core, HK)
        xt = np.ascontiguousarray(x[b].T.astype(np.float32))
        wq = np.ascontiguousarray(
            Wq[:, hk * G * D:(hk + 1) * G * D].astype(np.float32) * SCALE)
        wkv = np.ascontiguousarray(np.concatenate(
            [Wk[:, hk * D:(hk + 1) * D], Wv[:, hk * D:(hk + 1) * D]],
            axis=1).astype(np.float32))
        wo = np.ascontiguousarray(
            Wo[hk * G * D:(hk + 1) * G * D, :].astype(np.float32))
        in_maps.append({
            "xt": xt,
            "wq": wq,
            "wkv": wkv,
            "wo": wo,
            "coss": np.ascontiguousarray(rope_cos[b].astype(np.float32)),
            "sins": np.ascontiguousarray(rope_sin[b].astype(np.float32)),
            "mask": band,
            "eye": eye,
        })
    return in_maps


def _run(inputs, trace=False, **kw):
    if "nc" not in _compiled:
        _compiled["nc"] = _build()
    nc = _compiled["nc"]
    in_maps = _host_inputs(**inputs)
    res = run_bass_kernel_spmd(nc, in_maps, list(range(NCORES)), trace=trace, **kw)
    out = np.zeros((B, N, E), dtype=np.float32)
    for core in range(NCORES):
        b = core // HK
        out[b] += res.results[core]["out"]
    return out, res


def kernel(**inputs):
    out, _ = _run(inputs, trace=False)
    return out


# revision 3
# speedup vs baseline: 1.8724x; 1.8724x over previous
"""Grouped-query attention with sliding-window mask on 8 Trainium2 cores.

Sharding: core c handles (batch b = c // 4, kv-head group hk = c % 4).
Each core projects q (4 query heads) / k / v for its group, applies RoPE,
runs windowed attention, and computes a partial output projection
out_partial = attn_heads @ Wo[hk block].  The host sums the 4 partials
per batch.

Problem constants are hardcoded (B=2, N=2048, E=2048, H=16, G=4, D=128,
WIN=256); inputs arrive as full unsharded arrays.
"""

import sys

for _p in ("/opt/trn_rl_repo", "/opt/pypackages"):
    if _p not in sys.path:
        sys.path.insert(0, _p)

from contextlib import ExitStack

import numpy as np

import concourse.bacc as bacc
import concourse.mybir as mybir
import concourse.tile as tile
from concourse.bass_utils import run_bass_kernel_spmd

B, N, E = 2, 2048, 2048
H, G, WIN = 16, 4, 256
HK = H // G          # 4 kv heads
D = E // H           # 128
SCALE = D ** -0.5
NCORES = 8
P = 128
NT = N // P          # 16 n-tiles
EC = E // P          # 16 contraction chunks
HALF = N // 2
F32 = mybir.dt.float32
F32R = mybir.dt.float32r
MASK_VAL = -1.0e30

_compiled = {}


def _rope(nc, rtmp, dst, src, c_ap, s_ap, nblocks):
    """RoPE on [128, nblocks*128] (pairs adjacent along free dim).

    dst[2t]   = src[2t]*cos[t] - src[2t+1]*sin[t]
    dst[2t+1] = src[2t]*sin[t] + src[2t+1]*cos[t]
    """
    for blk in range(nblocks):
        b0 = blk * P
        x0 = src[:, b0 + 0:b0 + P:2]
        x1 = src[:, b0 + 1:b0 + P:2]
        t0 = rtmp.tile([P, D // 2], F32, tag="t0")
        t1 = rtmp.tile([P, D // 2], F32, tag="t1")
        nc.vector.tensor_mul(t0[:], x0, c_ap)
        nc.vector.tensor_mul(t1[:], x1, s_ap)
        nc.vector.tensor_sub(dst[:, b0 + 0:b0 + P:2], t0[:], t1[:])
        t2 = rtmp.tile([P, D // 2], F32, tag="t2")
        t3 = rtmp.tile([P, D // 2], F32, tag="t3")
        nc.vector.tensor_mul(t2[:], x0, s_ap)
        nc.vector.tensor_mul(t3[:], x1, c_ap)
        nc.vector.tensor_add(dst[:, b0 + 1:b0 + P:2], t2[:], t3[:])


def _build():
    nc = bacc.Bacc("TRN2", target_bir_lowering=False, debug=False)

    xt_d = nc.dram_tensor("xt", [E, N], F32R, kind="ExternalInput")
    wq_d = nc.dram_tensor("wq", [E, G * D], F32R, kind="ExternalInput")
    wkv_d = nc.dram_tensor("wkv", [E, 2 * D], F32R, kind="ExternalInput")
    wo_d = nc.dram_tensor("wo", [G * D, E], F32R, kind="ExternalInput")
    cos_d = nc.dram_tensor("coss", [N, D // 2], F32, kind="ExternalInput")
    sin_d = nc.dram_tensor("sins", [N, D // 2], F32, kind="ExternalInput")
    mask_d = nc.dram_tensor("mask", [P, 3 * P], F32, kind="ExternalInput")
    eye_d = nc.dram_tensor("eye", [P, P], F32R, kind="ExternalInput")
    out_d = nc.dram_tensor("out", [N, E], F32, kind="ExternalOutput")

    xt3 = xt_d.ap().rearrange("(c p) n -> c p n", p=P)
    wq3 = wq_d.ap().rearrange("(c p) m -> c p m", p=P)
    wkv3 = wkv_d.ap().rearrange("(c p) m -> c p m", p=P)
    wo3 = wo_d.ap().rearrange("(g p) e -> g p e", p=P)

    with tile.TileContext(nc) as tc, ExitStack() as top:
        pers = top.enter_context(tc.tile_pool(name="pers", bufs=1))
        qt_sb = pers.tile([P, G * N], F32R, tag="qt")      # per head g: [:, g*N + n]
        kt_sb = pers.tile([P, N], F32R, tag="kt")          # [d, n]
        v_sb = pers.tile([P, N], F32R, tag="v")            # tile t: [:, t*128] = v[t*128+p, d]
        cos_sb = pers.tile([P, NT * (D // 2)], F32, tag="cos")
        sin_sb = pers.tile([P, NT * (D // 2)], F32, tag="sin")
        mask_sb = pers.tile([P, 3 * P], F32, tag="mask")
        eye_sb = pers.tile([P, P], F32R, tag="eye")

        nc.sync.dma_start(
            cos_sb[:].rearrange("p (t d) -> p t d", d=D // 2),
            cos_d.ap().rearrange("(t p) d -> p t d", p=P),
        )
        nc.sync.dma_start(
            sin_sb[:].rearrange("p (t d) -> p t d", d=D // 2),
            sin_d.ap().rearrange("(t p) d -> p t d", p=P),
        )
        nc.sync.dma_start(mask_sb[:], mask_d.ap())
        nc.sync.dma_start(eye_sb[:], eye_d.ap())

        # ---------------- Phase A: projections + RoPE + transposes ---------
        with ExitStack() as pha:
            wq_pool = pha.enter_context(tc.tile_pool(name="wq", bufs=EC))
            wkv_pool = pha.enter_context(tc.tile_pool(name="wkv", bufs=EC))
            xt_pool = pha.enter_context(tc.tile_pool(name="xtp", bufs=EC))
            qrot_pool = pha.enter_context(tc.tile_pool(name="qrot", bufs=2))
            krot_pool = pha.enter_context(tc.tile_pool(name="krot", bufs=2))
            rtmp = pha.enter_context(tc.tile_pool(name="rtmp", bufs=4))
            qps_pool = pha.enter_context(
                tc.tile_pool(name="qps", bufs=2, space="PSUM"))
            kvps_pool = pha.enter_context(
                tc.tile_pool(name="kvps", bufs=2, space="PSUM"))
            trps_pool = pha.enter_context(
                tc.tile_pool(name="trps", bufs=2, space="PSUM"))

            wqs = [wq_pool.tile([P, G * D], F32R, tag="wq", name="wqt") for _ in range(EC)]
            wkvs = [wkv_pool.tile([P, 2 * D], F32R, tag="wkv", name="wkvt") for _ in range(EC)]
            for e in range(EC):
                nc.sync.dma_start(wqs[e][:], wq3[e])
                nc.sync.dma_start(wkvs[e][:], wkv3[e])

            for h in range(2):
                xts = [xt_pool.tile([P, HALF], F32R, tag="xt", name="xtt") for _ in range(EC)]
                for e in range(EC):
                    nc.sync.dma_start(xts[e][:], xt3[e][:, h * HALF:(h + 1) * HALF])

                for t in range(NT // 2):
                    T = h * (NT // 2) + t
                    q_ps = qps_pool.tile([P, G * D], F32, tag="qps")
                    kv_ps = kvps_pool.tile([P, 2 * D], F32, tag="kvps")
                    for e in range(EC):
                        nc.tensor.matmul(
                            q_ps[:], xts[e][:, t * P:(t + 1) * P], wqs[e][:],
                            start=(e == 0), stop=(e == EC - 1))
                    for e in range(EC):
                        nc.tensor.matmul(
                            kv_ps[:], xts[e][:, t * P:(t + 1) * P], wkvs[e][:],
                            start=(e == 0), stop=(e == EC - 1))

                    c_ap = cos_sb[:, T * (D // 2):(T + 1) * (D // 2)]
                    s_ap = sin_sb[:, T * (D // 2):(T + 1) * (D // 2)]
                    q_rot = qrot_pool.tile([P, G * D], F32R, tag="qrot")
                    k_rot = krot_pool.tile([P, D], F32R, tag="krot")
                    _rope(nc, rtmp, q_rot, q_ps, c_ap, s_ap, G)
                    _rope(nc, rtmp, k_rot, kv_ps, c_ap, s_ap, 1)
                    nc.scalar.copy(v_sb[:, T * P:(T + 1) * P], kv_ps[:, D:2 * D])

                    for g in range(G):
                        tq = trps_pool.tile([P, P], F32R, tag="trq")
                        nc.tensor.transpose(
                            tq[:], q_rot[:, g * P:(g + 1) * P], eye_sb[:])
                        nc.scalar.copy(
                            qt_sb[:, g * N + T * P: g * N + (T + 1) * P], tq[:])
                    tk = trps_pool.tile([P, P], F32R, tag="trq")
                    nc.tensor.transpose(tk[:], k_rot[:], eye_sb[:])
                    nc.scalar.copy(kt_sb[:, T * P:(T + 1) * P], tk[:])

        # ---------------- Phase B: attention + output projection -----------
        with ExitStack() as phb:
            wo_pool = phb.enter_context(tc.tile_pool(name="wo", bufs=G))
            sm_pool = phb.enter_context(tc.tile_pool(name="sm", bufs=3))
            stat_pool = phb.enter_context(tc.tile_pool(name="stat", bufs=6))
            pt_pool = phb.enter_context(tc.tile_pool(name="pt", bufs=2))
            ao_pool = phb.enter_context(tc.tile_pool(name="ao", bufs=2))
            osb_pool = phb.enter_context(tc.tile_pool(name="osb", bufs=2))
            sps_pool = phb.enter_context(
                tc.tile_pool(name="sps", bufs=2, space="PSUM"))
            ptps_pool = phb.enter_context(
                tc.tile_pool(name="ptps", bufs=1, space="PSUM"))
            aops_pool = phb.enter_context(
                tc.tile_pool(name="aops", bufs=1, space="PSUM"))
            wops_pool = phb.enter_context(
                tc.tile_pool(name="wops", bufs=1, space="PSUM"))

            wos = [wo_pool.tile([P, E], F32R, tag="wo", name="wot") for _ in range(G)]
            for g in range(G):
                nc.sync.dma_start(wos[g][:], wo3[g])

            for qt in range(NT):
                w0 = max(0, qt * P - WIN)
                wN = (qt + 1) * P - w0          # 128 / 256 / 384
                nk = wN // P
                ao_sb = ao_pool.tile([P, G * P], F32R, tag="ao")
                for g in range(G):
                    s_ps = sps_pool.tile([P, 3 * P], F32, tag="sps")
                    nc.tensor.matmul(
                        s_ps[:, :wN],
                        qt_sb[:, g * N + qt * P: g * N + (qt + 1) * P],
                        kt_sb[:, w0:w0 + wN],
                        start=True, stop=True)
                    s_sb = sm_pool.tile([P, 3 * P], F32, tag="ssb")
                    nc.vector.tensor_add(
                        s_sb[:, :wN], s_ps[:, :wN], mask_sb[:, 3 * P - wN:])
                    negm = stat_pool.tile([P, 1], F32, tag="negm")
                    nc.vector.reduce_max(
                        negm[:], s_sb[:, :wN], axis=mybir.AxisListType.X,
                        negate=True)
                    p_sb = sm_pool.tile([P, 3 * P], F32R, tag="psb")
                    denom = stat_pool.tile([P, 1], F32, tag="den")
                    nc.scalar.activation(
                        p_sb[:, :wN], s_sb[:, :wN],
                        mybir.ActivationFunctionType.Exp,
                        bias=negm[:], scale=1.0, accum_out=denom[:])
                    recip = stat_pool.tile([P, 1], F32, tag="rec")
                    nc.vector.reciprocal(recip[:], denom[:])
                    nc.vector.tensor_scalar_mul(p_sb[:, :wN], p_sb[:, :wN], recip[:])

                    pt_ps = ptps_pool.tile([P, 3 * P], F32R, tag="ptps")
                    for j in range(nk):
                        nc.tensor.transpose(
                            pt_ps[:, j * P:(j + 1) * P],
                            p_sb[:, j * P:(j + 1) * P], eye_sb[:])
                    pt_sb = pt_pool.tile([P, 3 * P], F32R, tag="ptsb")
                    nc.scalar.copy(pt_sb[:, :wN], pt_ps[:, :wN])

                    ao_ps = aops_pool.tile([P, P], F32, tag="aops")
                    for j in range(nk):
                        kb = w0 // P + j
                        nc.tensor.matmul(
                            ao_ps[:],
                            v_sb[:, kb * P:(kb + 1) * P],
                            pt_sb[:, j * P:(j + 1) * P],
                            start=(j == 0), stop=(j == nk - 1))
                    nc.scalar.copy(ao_sb[:, g * P:(g + 1) * P], ao_ps[:])

                wo_ps = wops_pool.tile([P, E], F32, tag="wops")
                for eb in range(4):
                    for g in range(G):
                        nc.tensor.matmul(
                            wo_ps[:, eb * 512:(eb + 1) * 512],
                            ao_sb[:, g * P:(g + 1) * P],
                            wos[g][:, eb * 512:(eb + 1) * 512],
                            start=(g == 0), stop=(g == G - 1))
                out_sb = osb_pool.tile([P, E], F32, tag="outsb")
                for eb in range(4):
                    nc.scalar.copy(
                        out_sb[:, eb * 512:(eb + 1) * 512],
                        wo_ps[:, eb * 512:(eb + 1) * 512])
                nc.sync.dma_start(out_d.ap()[qt * P:(qt + 1) * P, :], out_sb[:])

    nc.compile()
    return nc


def _host_inputs(x, rope_cos, rope_sin, Wq, Wk, Wv, Wo):
    """Build the 8 per-core input maps."""
    band = np.full((P, 3 * P), MASK_VAL, dtype=np.float32)
    r = np.arange(P)[:, None]
    c = np.arange(3 * P)[None, :]
    band[(c > r) & (c <= r + WIN)] = 0.0
    eye = np.eye(P, dtype=np.float32)

    in_maps = []
    for core in range(NCORES):
        b, hk = divmod(core, HK)
        xt = np.ascontiguousarray(x[b].T.astype(np.float32))
        wq = np.ascontiguousarray(
            Wq[:, hk * G * D:(hk + 1) * G * D].astype(np.float32) * SCALE)
        wkv = np.ascontiguousarray(np.concatenate(
            [Wk[:, hk * D:(hk + 1) * D], Wv[:, hk * D:(hk + 1) * D]],
            axis=1).astype(np.float32))
        wo = np.ascontiguousarray(
            Wo[hk * G * D:(hk + 1) * G * D, :].astype(np.float32))
        in_maps.append({
            "xt": xt,
            "wq": wq,
            "wkv": wkv,
            "wo": wo,
            "coss": np.ascontiguousarray(rope_cos[b].astype(np.float32)),
            "sins": np.ascontiguousarray(rope_sin[b].astype(np.float32)),
            "mask": band,
            "eye": eye,
        })
    return in_maps


def _run(inputs, trace=False, **kw):
    if "nc" not in _compiled:
        _compiled["nc"] = _build()
    nc = _compiled["nc"]
    in_maps = _host_inputs(**inputs)
    res = run_bass_kernel_spmd(nc, in_maps, list(range(NCORES)), trace=trace, **kw)
    out = np.zeros((B, N, E), dtype=np.float32)
    for core in range(NCORES):
        b = core // HK
        out[b] += res.results[core]["out"]
    return out, res


def kernel(**inputs):
    out, _ = _run(inputs, trace=False)
    return out


# revision 5
# speedup vs baseline: 1.9651x; 1.0495x over previous
"""Grouped-query attention with sliding-window mask on 8 Trainium2 cores.

Sharding: core c handles (batch b = c // 4, kv-head group hk = c % 4).
Each core projects q (4 query heads) / k / v for its group, applies RoPE,
runs windowed attention, and computes a partial output projection
out_partial = attn_heads @ Wo[hk block].  The host sums the 4 partials
per batch.

Attention runs in a scores-transposed layout: S^T[kj, (g,qi)] so all 4
heads share one N=512 moving operand per key tile (the kv head is shared
per group).  Softmax is max-free (scores are bounded ~|5|): the
denominator comes from a ones-vector matmul over exp(S^T), the reciprocal
is partition-broadcast back via a K=1 matmul, and normalization folds
into the PSUM->SBUF move of the attention output.

Matmul operands use float32r (TF32-like, ~1e-4 matmul error, 4x the fp32
rate at moving-dim >= 256).  Problem constants are hardcoded (B=2,
N=2048, E=2048, H=16, G=4, D=128, WIN=256).
"""

import sys

for _p in ("/opt/trn_rl_repo", "/opt/pypackages"):
    if _p not in sys.path:
        sys.path.insert(0, _p)

from contextlib import ExitStack

import numpy as np

import concourse.bacc as bacc
import concourse.bass as bass
import concourse.mybir as mybir
import concourse.tile as tile
from concourse.bass_utils import run_bass_kernel_spmd

B, N, E = 2, 2048, 2048
H, G, WIN = 16, 4, 256
HK = H // G          # 4 kv heads
D = E // H           # 128
SCALE = D ** -0.5
NCORES = 8
P = 128
NT = N // P          # 16 n-tiles
EC = E // P          # 16 contraction chunks
QTR = N // 4         # xT streamed in quarter-columns
F32 = mybir.dt.float32
F32R = mybir.dt.float32r
MASK_VAL = -1.0e30

_compiled = {}


def _rope(nc, rtmp, dst, src, c_ap, s_ap, nblocks):
    """RoPE on [128, nblocks*128] (pairs adjacent along free dim).

    dst[2t]   = src[2t]*cos[t] - src[2t+1]*sin[t]
    dst[2t+1] = src[2t]*sin[t] + src[2t+1]*cos[t]
    """
    for blk in range(nblocks):
        b0 = blk * P
        x0 = src[:, b0 + 0:b0 + P:2]
        x1 = src[:, b0 + 1:b0 + P:2]
        t0 = rtmp.tile([P, D // 2], F32, tag="t0", name="t0")
        t1 = rtmp.tile([P, D // 2], F32, tag="t1", name="t1")
        nc.vector.tensor_mul(t0[:], x0, c_ap)
        nc.vector.tensor_mul(t1[:], x1, s_ap)
        nc.vector.tensor_sub(dst[:, b0 + 0:b0 + P:2], t0[:], t1[:])
        t2 = rtmp.tile([P, D // 2], F32, tag="t2", name="t2")
        t3 = rtmp.tile([P, D // 2], F32, tag="t3", name="t3")
        nc.vector.tensor_mul(t2[:], x0, s_ap)
        nc.vector.tensor_mul(t3[:], x1, c_ap)
        nc.vector.tensor_add(dst[:, b0 + 1:b0 + P:2], t2[:], t3[:])


def _bcast_g(ap):
    """[128, 128] AP -> [128, G, 128] with a 0-step head dim."""
    return bass.AP(ap.tensor, ap.offset, [ap.ap[0], [0, G], ap.ap[1]])


def _build():
    nc = bacc.Bacc("TRN2", target_bir_lowering=False, debug=False)

    xt_d = nc.dram_tensor("xt", [E, N], F32R, kind="ExternalInput")
    wq_d = nc.dram_tensor("wq", [E, G * D], F32R, kind="ExternalInput")
    wkv_d = nc.dram_tensor("wkv", [E, 2 * D], F32R, kind="ExternalInput")
    wo_d = nc.dram_tensor("wo", [G * D, E], F32R, kind="ExternalInput")
    cos_d = nc.dram_tensor("coss", [N, D // 2], F32, kind="ExternalInput")
    sin_d = nc.dram_tensor("sins", [N, D // 2], F32, kind="ExternalInput")
    maskt_d = nc.dram_tensor("maskt", [P, 2 * P], F32, kind="ExternalInput")
    eye_d = nc.dram_tensor("eye", [P, P], F32R, kind="ExternalInput")
    onesr_d = nc.dram_tensor("onesr", [P, 1], F32R, kind="ExternalInput")
    out_d = nc.dram_tensor("out", [N, E], F32, kind="ExternalOutput")

    xt3 = xt_d.ap().rearrange("(c p) n -> c p n", p=P)
    wq3 = wq_d.ap().rearrange("(c p) m -> c p m", p=P)
    wkv3 = wkv_d.ap().rearrange("(c p) m -> c p m", p=P)
    wo3 = wo_d.ap().rearrange("(g p) e -> g p e", p=P)

    with tile.TileContext(nc) as tc, ExitStack() as top:
        pers = top.enter_context(tc.tile_pool(name="pers", bufs=1))
        # qT layout: [d, (qt, g, qi)] -> col = qt*512 + g*128 + qi
        qt_sb = pers.tile([P, G * N], F32R, tag="qt")
        kt_sb = pers.tile([P, N], F32R, tag="kt")          # [d, n]
        v_sb = pers.tile([P, N], F32R, tag="v")            # blk t: v[t*128+p, d]
        cos_sb = pers.tile([P, NT * (D // 2)], F32, tag="cos")
        sin_sb = pers.tile([P, NT * (D // 2)], F32, tag="sin")
        maskt_sb = pers.tile([P, 2 * P], F32, tag="maskt")
        eye_sb = pers.tile([P, P], F32R, tag="eye")
        ones_r = pers.tile([P, 1], F32R, tag="ones_r")     # denominator lhsT
        ones_f = pers.tile([1, P], F32, tag="ones_f")      # broadcast lhsT

        nc.sync.dma_start(
            cos_sb[:].rearrange("p (t d) -> p t d", d=D // 2),
            cos_d.ap().rearrange("(t p) d -> p t d", p=P),
        )
        nc.sync.dma_start(
            sin_sb[:].rearrange("p (t d) -> p t d", d=D // 2),
            sin_d.ap().rearrange("(t p) d -> p t d", p=P),
        )
        nc.sync.dma_start(maskt_sb[:], maskt_d.ap())
        nc.sync.dma_start(eye_sb[:], eye_d.ap())
        nc.sync.dma_start(ones_r[:], onesr_d.ap())
        nc.gpsimd.memset(ones_f[:], 1.0)

        wo_pool = top.enter_context(tc.tile_pool(name="wo", bufs=G))
        wos = [wo_pool.tile([P, E], F32R, tag="wo", name="wot") for _ in range(G)]

        # ---------------- Phase A: projections + RoPE + transposes ---------
        with ExitStack() as pha:
            wq_pool = pha.enter_context(tc.tile_pool(name="wq", bufs=EC))
            wkv_pool = pha.enter_context(tc.tile_pool(name="wkv", bufs=EC))
            xt_pool = pha.enter_context(tc.tile_pool(name="xtp", bufs=20))
            qrot_pool = pha.enter_context(tc.tile_pool(name="qrot", bufs=2))
            krot_pool = pha.enter_context(tc.tile_pool(name="krot", bufs=2))
            rtmp = pha.enter_context(tc.tile_pool(name="rtmp", bufs=4))
            qps_pool = pha.enter_context(
                tc.tile_pool(name="qps", bufs=2, space="PSUM"))
            kvps_pool = pha.enter_context(
                tc.tile_pool(name="kvps", bufs=2, space="PSUM"))
            trps_pool = pha.enter_context(
                tc.tile_pool(name="trps", bufs=2, space="PSUM"))

            wqs = [wq_pool.tile([P, G * D], F32R, tag="wq", name="wqt")
                   for _ in range(EC)]
            wkvs = [wkv_pool.tile([P, 2 * D], F32R, tag="wkv", name="wkvt")
                    for _ in range(EC)]
            for e in range(EC):
                nc.sync.dma_start(wqs[e][:], wq3[e])
                nc.sync.dma_start(wkvs[e][:], wkv3[e])

            for qtr in range(4):
                xts = [xt_pool.tile([P, QTR], F32R, tag="xt", name="xtt")
                       for _ in range(EC)]
                for e in range(EC):
                    nc.sync.dma_start(
                        xts[e][:], xt3[e][:, qtr * QTR:(qtr + 1) * QTR])
                if qtr == 0:
                    # wo is needed only in phase B; load it behind the first
                    # xT quarter so it never stalls the PE later.
                    for g in range(G):
                        nc.sync.dma_start(wos[g][:], wo3[g])

                for t in range(4):
                    T = qtr * 4 + t
                    q_ps = qps_pool.tile([P, G * D], F32, tag="qps")
                    kv_ps = kvps_pool.tile([P, 2 * D], F32, tag="kvps")
                    for e in range(EC):
                        nc.tensor.matmul(
                            q_ps[:], xts[e][:, t * P:(t + 1) * P], wqs[e][:],
                            start=(e == 0), stop=(e == EC - 1))
                    for e in range(EC):
                        nc.tensor.matmul(
                            kv_ps[:], xts[e][:, t * P:(t + 1) * P], wkvs[e][:],
                            start=(e == 0), stop=(e == EC - 1))

                    c_ap = cos_sb[:, T * (D // 2):(T + 1) * (D // 2)]
                    s_ap = sin_sb[:, T * (D // 2):(T + 1) * (D // 2)]
                    q_rot = qrot_pool.tile([P, G * D], F32R, tag="qrot")
                    k_rot = krot_pool.tile([P, D], F32R, tag="krot")
                    _rope(nc, rtmp, q_rot, q_ps, c_ap, s_ap, G)
                    _rope(nc, rtmp, k_rot, kv_ps, c_ap, s_ap, 1)
                    nc.scalar.copy(v_sb[:, T * P:(T + 1) * P], kv_ps[:, D:2 * D])

                    for g in range(G):
                        tq = trps_pool.tile([P, P], F32R, tag="trq", name="trq")
                        nc.tensor.transpose(
                            tq[:], q_rot[:, g * P:(g + 1) * P], eye_sb[:])
                        nc.scalar.copy(
                            qt_sb[:, T * 4 * P + g * P: T * 4 * P + (g + 1) * P],
                            tq[:])
                    tk = trps_pool.tile([P, P], F32R, tag="trq", name="trk")
                    nc.tensor.transpose(tk[:], k_rot[:], eye_sb[:])
                    nc.scalar.copy(kt_sb[:, T * P:(T + 1) * P], tk[:])

        # ---------------- Phase B: attention + output projection -----------
        with ExitStack() as phb:
            ex_pool = phb.enter_context(tc.tile_pool(name="ex", bufs=2))
            smm_pool = phb.enter_context(tc.tile_pool(name="smm", bufs=2))
            stat_pool = phb.enter_context(tc.tile_pool(name="stat", bufs=4))
            bcs_pool = phb.enter_context(tc.tile_pool(name="bcs", bufs=2))
            ao_pool = phb.enter_context(tc.tile_pool(name="aosb", bufs=2))
            osb_pool = phb.enter_context(tc.tile_pool(name="osb", bufs=2))
            sps_pool = phb.enter_context(
                tc.tile_pool(name="sps", bufs=2, space="PSUM"))
            denps_pool = phb.enter_context(
                tc.tile_pool(name="denps", bufs=1, space="PSUM"))
            aops_pool = phb.enter_context(
                tc.tile_pool(name="aops", bufs=2, space="PSUM"))
            bcps_pool = phb.enter_context(
                tc.tile_pool(name="bcps", bufs=1, space="PSUM"))
            wops_pool = phb.enter_context(
                tc.tile_pool(name="wops", bufs=2, space="PSUM"))

            W = G * P  # 512: (g, qi) moving width
            for qt in range(NT):
                nk = min(qt, 2) + 1
                kb0 = qt - (nk - 1)
                exps = ex_pool.tile([P, 3 * W], F32R, tag="exps")
                den_ps = denps_pool.tile([1, W], F32, tag="denps")
                for j in range(nk):
                    kb = kb0 + j
                    dabs = kb - qt          # -2, -1, or 0
                    st_ps = sps_pool.tile([P, W], F32, tag="stps")
                    nc.tensor.matmul(
                        st_ps[:],
                        kt_sb[:, kb * P:(kb + 1) * P],
                        qt_sb[:, qt * W:(qt + 1) * W],
                        start=True, stop=True)
                    eblk = exps[:, j * W:(j + 1) * W]
                    if dabs == -1:
                        nc.scalar.activation(
                            eblk, st_ps[:], mybir.ActivationFunctionType.Exp)
                    else:
                        mblk = maskt_sb[:, 0:P] if dabs == -2 \
                            else maskt_sb[:, P:2 * P]
                        st_sb = smm_pool.tile([P, W], F32, tag="stsb")
                        nc.vector.tensor_add(
                            st_sb[:].rearrange("p (g q) -> p g q", g=G),
                            st_ps[:].rearrange("p (g q) -> p g q", g=G),
                            _bcast_g(mblk))
                        nc.scalar.activation(
                            eblk, st_sb[:], mybir.ActivationFunctionType.Exp)
                    nc.tensor.matmul(
                        den_ps[:], ones_r[:, :1], eblk,
                        start=(j == 0), stop=(j == nk - 1))

                recip = stat_pool.tile([1, W], F32, tag="recip")
                nc.vector.reciprocal(recip[:], den_ps[:])
                bc_ps = bcps_pool.tile([P, W], F32, tag="bcps")
                nc.tensor.matmul(bc_ps[:], ones_f[:1, :], recip[:],
                                 start=True, stop=True)
                bc_sb = bcs_pool.tile([P, W], F32, tag="bcsb")
                nc.scalar.copy(bc_sb[:], bc_ps[:])

                ao_ps = aops_pool.tile([P, W], F32, tag="aops")
                for j in range(nk):
                    kb = kb0 + j
                    nc.tensor.matmul(
                        ao_ps[:],
                        v_sb[:, kb * P:(kb + 1) * P],
                        exps[:, j * W:(j + 1) * W],
                        start=(j == 0), stop=(j == nk - 1))
                ao_sb = ao_pool.tile([P, W], F32R, tag="aosb")
                nc.vector.tensor_mul(ao_sb[:], ao_ps[:], bc_sb[:])

                out_sb = osb_pool.tile([P, E], F32, tag="outsb")
                for eb in range(4):
                    wo_ps = wops_pool.tile([P, 512], F32, tag="wops")
                    for g in range(G):
                        nc.tensor.matmul(
                            wo_ps[:],
                            ao_sb[:, g * P:(g + 1) * P],
                            wos[g][:, eb * 512:(eb + 1) * 512],
                            start=(g == 0), stop=(g == G - 1))
                    if eb % 2 == 0:
                        nc.scalar.copy(out_sb[:, eb * 512:(eb + 1) * 512],
                                       wo_ps[:])
                    else:
                        nc.vector.tensor_copy(out_sb[:, eb * 512:(eb + 1) * 512],
                                              wo_ps[:])
                nc.sync.dma_start(out_d.ap()[qt * P:(qt + 1) * P, :], out_sb[:])

    nc.compile()
    return nc


def _host_inputs(x, rope_cos, rope_sin, Wq, Wk, Wv, Wo):
    """Build the 8 per-core input maps."""
    band = np.full((P, 3 * P), MASK_VAL, dtype=np.float32)
    r = np.arange(P)[:, None]
    c = np.arange(3 * P)[None, :]
    band[(c > r) & (c <= r + WIN)] = 0.0
    # transposed mask blocks: [:, :128] for key-tile offset -2,
    # [:, 128:] (causal) for offset 0
    maskt = np.ascontiguousarray(np.concatenate(
        [band[:, 0:P].T, band[:, 2 * P:3 * P].T], axis=1))
    eye = np.eye(P, dtype=np.float32)

    in_maps = []
    for core in range(NCORES):
        b, hk = divmod(core, HK)
        xt = np.ascontiguousarray(x[b].T.astype(np.float32))
        wq = np.ascontiguousarray(
            Wq[:, hk * G * D:(hk + 1) * G * D].astype(np.float32) * SCALE)
        wkv = np.ascontiguousarray(np.concatenate(
            [Wk[:, hk * D:(hk + 1) * D], Wv[:, hk * D:(hk + 1) * D]],
            axis=1).astype(np.float32))
        wo = np.ascontiguousarray(
            Wo[hk * G * D:(hk + 1) * G * D, :].astype(np.float32))
        in_maps.append({
            "xt": xt,
            "wq": wq,
            "wkv": wkv,
            "wo": wo,
            "coss": np.ascontiguousarray(rope_cos[b].astype(np.float32)),
            "sins": np.ascontiguousarray(rope_sin[b].astype(np.float32)),
            "maskt": maskt,
            "eye": eye,
            "onesr": np.ones((P, 1), dtype=np.float32),
        })
    return in_maps


def _run(inputs, trace=False, **kw):
    if "nc" not in _compiled:
        _compiled["nc"] = _build()
    nc = _compiled["nc"]
    in_maps = _host_inputs(**inputs)
    res = run_bass_kernel_spmd(nc, in_maps, list(range(NCORES)), trace=trace, **kw)
    out = np.zeros((B, N, E), dtype=np.float32)
    for core in range(NCORES):
        b = core // HK
        out[b] += res.results[core]["out"]
    return out, res


def kernel(**inputs):
    out, _ = _run(inputs, trace=False)
    return out


# revision 6
# speedup vs baseline: 2.5016x; 1.2730x over previous
"""Grouped-query attention with sliding-window mask on 8 Trainium2 cores.

Sharding: core c handles (batch b = c // 4, kv-head group hk = c % 4).
Each core projects q (4 query heads) / k / v for its group, applies RoPE,
runs windowed attention, and computes a partial output projection
out_partial = attn_heads @ Wo[hk block].  The host sums the 4 partials
per batch.

Attention runs in a scores-transposed layout: S^T[kj, (g,qi)] so all 4
heads share one N=512 moving operand per key tile (the kv head is shared
per group).  Softmax is max-free (scores are bounded ~|5|): the
denominator comes from a ones-vector matmul over exp(S^T), the reciprocal
is partition-broadcast back via a K=1 matmul, and normalization folds
into the PSUM->SBUF move of the attention output.

Matmul operands use float32r (TF32-like, ~1e-4 matmul error, 4x the fp32
rate at moving-dim >= 256).  Problem constants are hardcoded (B=2,
N=2048, E=2048, H=16, G=4, D=128, WIN=256).
"""

import sys

for _p in ("/opt/trn_rl_repo", "/opt/pypackages"):
    if _p not in sys.path:
        sys.path.insert(0, _p)

from contextlib import ExitStack

import numpy as np

import concourse.bacc as bacc
import concourse.bass as bass
import concourse.mybir as mybir
import concourse.tile as tile
from concourse.bass_utils import run_bass_kernel_spmd

B, N, E = 2, 2048, 2048
H, G, WIN = 16, 4, 256
HK = H // G          # 4 kv heads
D = E // H           # 128
SCALE = D ** -0.5
NCORES = 8
P = 128
NT = N // P          # 16 n-tiles
EC = E // P          # 16 contraction chunks
QTR = N // 4         # xT streamed in quarter-columns
F32 = mybir.dt.float32
F32R = mybir.dt.float32r
MASK_VAL = -1.0e30

_compiled = {}


def _rope(nc, rtmp, dst, src, c_ap, s_ap, nblocks):
    """RoPE on [128, nblocks*128] (pairs adjacent along free dim), all
    head-blocks in one strided op set.

    dst[2t]   = src[2t]*cos[t] - src[2t+1]*sin[t]
    dst[2t+1] = src[2t]*sin[t] + src[2t+1]*cos[t]
    """
    sb = src[:]
    db = dst[:]
    hd = D // 2
    x0 = bass.AP(sb.tensor, sb.offset + 0, [sb.ap[0], [P, nblocks], [2, hd]])
    x1 = bass.AP(sb.tensor, sb.offset + 1, [sb.ap[0], [P, nblocks], [2, hd]])
    d0 = bass.AP(db.tensor, db.offset + 0, [db.ap[0], [P, nblocks], [2, hd]])
    d1 = bass.AP(db.tensor, db.offset + 1, [db.ap[0], [P, nblocks], [2, hd]])
    cb = c_ap
    c3 = bass.AP(cb.tensor, cb.offset, [cb.ap[0], [0, nblocks], cb.ap[1]])
    s3 = bass.AP(s_ap.tensor, s_ap.offset, [s_ap.ap[0], [0, nblocks], s_ap.ap[1]])
    t0 = rtmp.tile([P, nblocks * hd], F32, tag="t0", name="t0")
    t1 = rtmp.tile([P, nblocks * hd], F32, tag="t1", name="t1")
    t0v = t0[:].rearrange("p (b d) -> p b d", d=hd)
    t1v = t1[:].rearrange("p (b d) -> p b d", d=hd)
    nc.vector.tensor_mul(t0v, x0, c3)
    nc.vector.tensor_mul(t1v, x1, s3)
    nc.vector.tensor_sub(d0, t0v, t1v)
    t2 = rtmp.tile([P, nblocks * hd], F32, tag="t2", name="t2")
    t3 = rtmp.tile([P, nblocks * hd], F32, tag="t3", name="t3")
    t2v = t2[:].rearrange("p (b d) -> p b d", d=hd)
    t3v = t3[:].rearrange("p (b d) -> p b d", d=hd)
    nc.vector.tensor_mul(t2v, x0, s3)
    nc.vector.tensor_mul(t3v, x1, c3)
    nc.vector.tensor_add(d1, t2v, t3v)


def _bcast_g(ap):
    """[128, 128] AP -> [128, G, 128] with a 0-step head dim."""
    return bass.AP(ap.tensor, ap.offset, [ap.ap[0], [0, G], ap.ap[1]])


def _build():
    nc = bacc.Bacc("TRN2", target_bir_lowering=False, debug=False)

    xt_d = nc.dram_tensor("xt", [E, N], F32R, kind="ExternalInput")
    wq_d = nc.dram_tensor("wq", [E, G * D], F32R, kind="ExternalInput")
    wkv_d = nc.dram_tensor("wkv", [E, 2 * D], F32R, kind="ExternalInput")
    wo_d = nc.dram_tensor("wo", [G * D, E], F32R, kind="ExternalInput")
    cos_d = nc.dram_tensor("coss", [N, D // 2], F32, kind="ExternalInput")
    sin_d = nc.dram_tensor("sins", [N, D // 2], F32, kind="ExternalInput")
    maskt_d = nc.dram_tensor("maskt", [P, 2 * P], F32, kind="ExternalInput")
    eye_d = nc.dram_tensor("eye", [P, P], F32R, kind="ExternalInput")
    onesr_d = nc.dram_tensor("onesr", [P, 1], F32R, kind="ExternalInput")
    out_d = nc.dram_tensor("out", [N, E], F32, kind="ExternalOutput")

    xt3 = xt_d.ap().rearrange("(c p) n -> c p n", p=P)
    wq3 = wq_d.ap().rearrange("(c p) m -> c p m", p=P)
    wkv3 = wkv_d.ap().rearrange("(c p) m -> c p m", p=P)
    wo3 = wo_d.ap().rearrange("(g p) e -> g p e", p=P)

    with tile.TileContext(nc) as tc, ExitStack() as top:
        pers = top.enter_context(tc.tile_pool(name="pers", bufs=1))
        # qT layout: [d, (qt, g, qi)] -> col = qt*512 + g*128 + qi
        qt_sb = pers.tile([P, G * N], F32R, tag="qt")
        kt_sb = pers.tile([P, N], F32R, tag="kt")          # [d, n]
        v_sb = pers.tile([P, N], F32R, tag="v")            # blk t: v[t*128+p, d]
        cos_sb = pers.tile([P, NT * (D // 2)], F32, tag="cos")
        sin_sb = pers.tile([P, NT * (D // 2)], F32, tag="sin")
        maskt_sb = pers.tile([P, 2 * P], F32, tag="maskt")
        eye_sb = pers.tile([P, P], F32R, tag="eye")
        ones_r = pers.tile([P, 1], F32R, tag="ones_r")     # denominator lhsT

        nc.sync.dma_start(
            cos_sb[:].rearrange("p (t d) -> p t d", d=D // 2),
            cos_d.ap().rearrange("(t p) d -> p t d", p=P),
        )
        nc.sync.dma_start(
            sin_sb[:].rearrange("p (t d) -> p t d", d=D // 2),
            sin_d.ap().rearrange("(t p) d -> p t d", p=P),
        )
        nc.sync.dma_start(maskt_sb[:], maskt_d.ap())
        nc.sync.dma_start(eye_sb[:], eye_d.ap())
        nc.sync.dma_start(ones_r[:], onesr_d.ap())

        wo_pool = top.enter_context(tc.tile_pool(name="wo", bufs=G))
        wos = [wo_pool.tile([P, E], F32R, tag="wo", name="wot") for _ in range(G)]

        # ---------------- Phase A: projections + RoPE + transposes ---------
        with ExitStack() as pha:
            wq_pool = pha.enter_context(tc.tile_pool(name="wq", bufs=EC))
            wkv_pool = pha.enter_context(tc.tile_pool(name="wkv", bufs=EC))
            xt_pool = pha.enter_context(tc.tile_pool(name="xtp", bufs=20))
            qrot_pool = pha.enter_context(tc.tile_pool(name="qrot", bufs=2))
            krot_pool = pha.enter_context(tc.tile_pool(name="krot", bufs=2))
            rtmp = pha.enter_context(tc.tile_pool(name="rtmp", bufs=4))
            qps_pool = pha.enter_context(
                tc.tile_pool(name="qps", bufs=2, space="PSUM"))
            kvps_pool = pha.enter_context(
                tc.tile_pool(name="kvps", bufs=2, space="PSUM"))
            trps_pool = pha.enter_context(
                tc.tile_pool(name="trps", bufs=2, space="PSUM"))

            wqs = [wq_pool.tile([P, G * D], F32R, tag="wq", name="wqt")
                   for _ in range(EC)]
            wkvs = [wkv_pool.tile([P, 2 * D], F32R, tag="wkv", name="wkvt")
                    for _ in range(EC)]
            for e in range(EC):
                nc.sync.dma_start(wqs[e][:], wq3[e])
                nc.sync.dma_start(wkvs[e][:], wkv3[e])

            for qtr in range(4):
                xts = [xt_pool.tile([P, QTR], F32R, tag="xt", name="xtt")
                       for _ in range(EC)]
                for e in range(EC):
                    nc.sync.dma_start(
                        xts[e][:], xt3[e][:, qtr * QTR:(qtr + 1) * QTR])
                if qtr == 1:
                    # wo is needed only in phase B; load it behind the first
                    # xT quarter so it never stalls the PE later.
                    for g in range(G):
                        nc.sync.dma_start(wos[g][:], wo3[g])

                for t in range(4):
                    T = qtr * 4 + t
                    q_ps = qps_pool.tile([P, G * D], F32, tag="qps")
                    kv_ps = kvps_pool.tile([P, 2 * D], F32, tag="kvps")
                    for e in range(EC):
                        nc.tensor.matmul(
                            q_ps[:], xts[e][:, t * P:(t + 1) * P], wqs[e][:],
                            start=(e == 0), stop=(e == EC - 1))
                    for e in range(EC):
                        nc.tensor.matmul(
                            kv_ps[:], xts[e][:, t * P:(t + 1) * P], wkvs[e][:],
                            start=(e == 0), stop=(e == EC - 1))

                    c_ap = cos_sb[:, T * (D // 2):(T + 1) * (D // 2)]
                    s_ap = sin_sb[:, T * (D // 2):(T + 1) * (D // 2)]
                    q_rot = qrot_pool.tile([P, G * D], F32R, tag="qrot")
                    k_rot = krot_pool.tile([P, D], F32R, tag="krot")
                    _rope(nc, rtmp, q_rot, q_ps, c_ap, s_ap, G)
                    _rope(nc, rtmp, k_rot, kv_ps, c_ap, s_ap, 1)
                    nc.scalar.copy(v_sb[:, T * P:(T + 1) * P], kv_ps[:, D:2 * D])

                    for g in range(G):
                        tq = trps_pool.tile([P, P], F32R, tag="trq", name="trq")
                        nc.tensor.transpose(
                            tq[:], q_rot[:, g * P:(g + 1) * P], eye_sb[:])
                        nc.scalar.copy(
                            qt_sb[:, T * 4 * P + g * P: T * 4 * P + (g + 1) * P],
                            tq[:])
                    tk = trps_pool.tile([P, P], F32R, tag="trq", name="trk")
                    nc.tensor.transpose(tk[:], k_rot[:], eye_sb[:])
                    nc.scalar.copy(kt_sb[:, T * P:(T + 1) * P], tk[:])

        # ---------------- Phase B: attention + output projection -----------
        with ExitStack() as phb:
            ex_pool = phb.enter_context(tc.tile_pool(name="ex", bufs=2))
            smm_pool = phb.enter_context(tc.tile_pool(name="smm", bufs=2))
            stat_pool = phb.enter_context(tc.tile_pool(name="stat", bufs=4))
            bcs_pool = phb.enter_context(tc.tile_pool(name="bcs", bufs=2))
            ao_pool = phb.enter_context(tc.tile_pool(name="aosb", bufs=2))
            osb_pool = phb.enter_context(tc.tile_pool(name="osb", bufs=2))
            sps_pool = phb.enter_context(
                tc.tile_pool(name="sps", bufs=2, space="PSUM"))
            denps_pool = phb.enter_context(
                tc.tile_pool(name="denps", bufs=1, space="PSUM"))
            aops_pool = phb.enter_context(
                tc.tile_pool(name="aops", bufs=2, space="PSUM"))
            wops_pool = phb.enter_context(
                tc.tile_pool(name="wops", bufs=3, space="PSUM"))

            W = G * P  # 512: (g, qi) moving width
            for qt in range(NT):
                nk = min(qt, 2) + 1
                kb0 = qt - (nk - 1)
                exps = ex_pool.tile([P, 3 * W], F32R, tag="exps")
                den_ps = denps_pool.tile([1, W], F32, tag="denps")
                for j in range(nk):
                    kb = kb0 + j
                    dabs = kb - qt          # -2, -1, or 0
                    st_ps = sps_pool.tile([P, W], F32, tag="stps")
                    nc.tensor.matmul(
                        st_ps[:],
                        kt_sb[:, kb * P:(kb + 1) * P],
                        qt_sb[:, qt * W:(qt + 1) * W],
                        start=True, stop=True)
                    eblk = exps[:, j * W:(j + 1) * W]
                    if dabs == -1:
                        nc.scalar.activation(
                            eblk, st_ps[:], mybir.ActivationFunctionType.Exp)
                    else:
                        mblk = maskt_sb[:, 0:P] if dabs == -2 \
                            else maskt_sb[:, P:2 * P]
                        st_sb = smm_pool.tile([P, W], F32, tag="stsb")
                        nc.vector.tensor_add(
                            st_sb[:].rearrange("p (g q) -> p g q", g=G),
                            st_ps[:].rearrange("p (g q) -> p g q", g=G),
                            _bcast_g(mblk))
                        nc.scalar.activation(
                            eblk, st_sb[:], mybir.ActivationFunctionType.Exp)
                    nc.tensor.matmul(
                        den_ps[:], ones_r[:, :1], eblk,
                        start=(j == 0), stop=(j == nk - 1))

                recip = stat_pool.tile([1, W], F32, tag="recip")
                nc.vector.reciprocal_approx_fast(recip[:], den_ps[:])
                bc_sb = bcs_pool.tile([P, W], F32, tag="bcsb")
                nc.gpsimd.partition_broadcast(bc_sb[:], recip[:])

                ao_ps = aops_pool.tile([P, W], F32, tag="aops")
                for j in range(nk):
                    kb = kb0 + j
                    nc.tensor.matmul(
                        ao_ps[:],
                        v_sb[:, kb * P:(kb + 1) * P],
                        exps[:, j * W:(j + 1) * W],
                        start=(j == 0), stop=(j == nk - 1))
                ao_sb = ao_pool.tile([P, W], F32R, tag="aosb")
                nc.vector.tensor_mul(ao_sb[:], ao_ps[:], bc_sb[:])

                out_sb = osb_pool.tile([P, E], F32, tag="outsb")
                for eb in range(4):
                    wo_ps = wops_pool.tile([P, 512], F32, tag="wops")
                    for g in range(G):
                        nc.tensor.matmul(
                            wo_ps[:],
                            ao_sb[:, g * P:(g + 1) * P],
                            wos[g][:, eb * 512:(eb + 1) * 512],
                            start=(g == 0), stop=(g == G - 1))
                    if eb % 2 == 0:
                        nc.scalar.copy(out_sb[:, eb * 512:(eb + 1) * 512],
                                       wo_ps[:])
                    else:
                        nc.vector.tensor_copy(out_sb[:, eb * 512:(eb + 1) * 512],
                                              wo_ps[:])
                nc.sync.dma_start(out_d.ap()[qt * P:(qt + 1) * P, :], out_sb[:])

    nc.compile()
    return nc


def _host_inputs(x, rope_cos, rope_sin, Wq, Wk, Wv, Wo):
    """Build the 8 per-core input maps."""
    band = np.full((P, 3 * P), MASK_VAL, dtype=np.float32)
    r = np.arange(P)[:, None]
    c = np.arange(3 * P)[None, :]
    band[(c > r) & (c <= r + WIN)] = 0.0
    # transposed mask blocks: [:, :128] for key-tile offset -2,
    # [:, 128:] (causal) for offset 0
    maskt = np.ascontiguousarray(np.concatenate(
        [band[:, 0:P].T, band[:, 2 * P:3 * P].T], axis=1))
    eye = np.eye(P, dtype=np.float32)

    in_maps = []
    for core in range(NCORES):
        b, hk = divmod(core, HK)
        xt = np.ascontiguousarray(x[b].T.astype(np.float32))
        wq = np.ascontiguousarray(
            Wq[:, hk * G * D:(hk + 1) * G * D].astype(np.float32) * SCALE)
        wkv = np.ascontiguousarray(np.concatenate(
            [Wk[:, hk * D:(hk + 1) * D], Wv[:, hk * D:(hk + 1) * D]],
            axis=1).astype(np.float32))
        wo = np.ascontiguousarray(
            Wo[hk * G * D:(hk + 1) * G * D, :].astype(np.float32))
        in_maps.append({
            "xt": xt,
            "wq": wq,
            "wkv": wkv,
            "wo": wo,
            "coss": np.ascontiguousarray(rope_cos[b].astype(np.float32)),
            "sins": np.ascontiguousarray(rope_sin[b].astype(np.float32)),
            "maskt": maskt,
            "eye": eye,
            "onesr": np.ones((P, 1), dtype=np.float32),
        })
    return in_maps


def _run(inputs, trace=False, **kw):
    if "nc" not in _compiled:
        _compiled["nc"] = _build()
    nc = _compiled["nc"]
    in_maps = _host_inputs(**inputs)
    res = run_bass_kernel_spmd(nc, in_maps, list(range(NCORES)), trace=trace, **kw)
    out = np.zeros((B, N, E), dtype=np.float32)
    for core in range(NCORES):
        b = core // HK
        out[b] += res.results[core]["out"]
    return out, res


def kernel(**inputs):
    out, _ = _run(inputs, trace=False)
    return out


# revision 7
# speedup vs baseline: 2.5482x; 1.0186x over previous
"""Grouped-query attention with sliding-window mask on 8 Trainium2 cores.

Sharding: core c handles (batch b = c // 4, kv-head group hk = c % 4).
Each core projects q (4 query heads) / k / v for its group, applies RoPE,
runs windowed attention, and computes a partial output projection
out_partial = attn_heads @ Wo[hk block].  The host sums the 4 partials
per batch.

Attention runs in a scores-transposed layout: S^T[kj, (g,qi)] so all 4
heads share one N=512 moving operand per key tile (the kv head is shared
per group).  Softmax is max-free (scores are bounded ~|5|): the
denominator comes from a ones-vector matmul over exp(S^T), the reciprocal
is partition-broadcast back via a K=1 matmul, and normalization folds
into the PSUM->SBUF move of the attention output.

Matmul operands use float32r (TF32-like, ~1e-4 matmul error, 4x the fp32
rate at moving-dim >= 256).  Problem constants are hardcoded (B=2,
N=2048, E=2048, H=16, G=4, D=128, WIN=256).
"""

import sys

for _p in ("/opt/trn_rl_repo", "/opt/pypackages"):
    if _p not in sys.path:
        sys.path.insert(0, _p)

from contextlib import ExitStack

import numpy as np

import concourse.bacc as bacc
import concourse.bass as bass
import concourse.mybir as mybir
import concourse.tile as tile
from concourse.bass_utils import run_bass_kernel_spmd

B, N, E = 2, 2048, 2048
H, G, WIN = 16, 4, 256
HK = H // G          # 4 kv heads
D = E // H           # 128
SCALE = D ** -0.5
NCORES = 8
P = 128
NT = N // P          # 16 n-tiles
EC = E // P          # 16 contraction chunks
QTR = N // 4         # xT streamed in quarter-columns
F32 = mybir.dt.float32
F32R = mybir.dt.float32r
MASK_VAL = -1.0e30

_compiled = {}


def _rope(nc, rtmp, dst, src, c_ap, s_ap, nblocks):
    """RoPE on [128, nblocks*128] (pairs adjacent along free dim), all
    head-blocks in one strided op set.

    dst[2t]   = src[2t]*cos[t] - src[2t+1]*sin[t]
    dst[2t+1] = src[2t]*sin[t] + src[2t+1]*cos[t]
    """
    sb = src[:]
    db = dst[:]
    hd = D // 2
    x0 = bass.AP(sb.tensor, sb.offset + 0, [sb.ap[0], [P, nblocks], [2, hd]])
    x1 = bass.AP(sb.tensor, sb.offset + 1, [sb.ap[0], [P, nblocks], [2, hd]])
    d0 = bass.AP(db.tensor, db.offset + 0, [db.ap[0], [P, nblocks], [2, hd]])
    d1 = bass.AP(db.tensor, db.offset + 1, [db.ap[0], [P, nblocks], [2, hd]])
    cb = c_ap
    c3 = bass.AP(cb.tensor, cb.offset, [cb.ap[0], [0, nblocks], cb.ap[1]])
    s3 = bass.AP(s_ap.tensor, s_ap.offset, [s_ap.ap[0], [0, nblocks], s_ap.ap[1]])
    t0 = rtmp.tile([P, nblocks * hd], F32, tag="t0", name="t0")
    t1 = rtmp.tile([P, nblocks * hd], F32, tag="t1", name="t1")
    t0v = t0[:].rearrange("p (b d) -> p b d", d=hd)
    t1v = t1[:].rearrange("p (b d) -> p b d", d=hd)
    nc.vector.tensor_mul(t0v, x0, c3)
    nc.vector.tensor_mul(t1v, x1, s3)
    nc.vector.tensor_sub(d0, t0v, t1v)
    t2 = rtmp.tile([P, nblocks * hd], F32, tag="t2", name="t2")
    t3 = rtmp.tile([P, nblocks * hd], F32, tag="t3", name="t3")
    t2v = t2[:].rearrange("p (b d) -> p b d", d=hd)
    t3v = t3[:].rearrange("p (b d) -> p b d", d=hd)
    nc.vector.tensor_mul(t2v, x0, s3)
    nc.vector.tensor_mul(t3v, x1, c3)
    nc.vector.tensor_add(d1, t2v, t3v)


def _bcast_g(ap):
    """[128, 128] AP -> [128, G, 128] with a 0-step head dim."""
    return bass.AP(ap.tensor, ap.offset, [ap.ap[0], [0, G], ap.ap[1]])


def _build():
    nc = bacc.Bacc("TRN2", target_bir_lowering=False, debug=False)

    xt_d = nc.dram_tensor("xt", [E, N], F32R, kind="ExternalInput")
    wq_d = nc.dram_tensor("wq", [E, G * D], F32R, kind="ExternalInput")
    wkv_d = nc.dram_tensor("wkv", [E, 2 * D], F32R, kind="ExternalInput")
    wo_d = nc.dram_tensor("wo", [G * D, E], F32R, kind="ExternalInput")
    cos_d = nc.dram_tensor("coss", [N, D // 2], F32, kind="ExternalInput")
    sin_d = nc.dram_tensor("sins", [N, D // 2], F32, kind="ExternalInput")
    maskt_d = nc.dram_tensor("maskt", [P, 2 * P], F32, kind="ExternalInput")
    eye_d = nc.dram_tensor("eye", [P, P], F32R, kind="ExternalInput")
    onesr_d = nc.dram_tensor("onesr", [P, 1], F32R, kind="ExternalInput")
    out_d = nc.dram_tensor("out", [N, E], F32, kind="ExternalOutput")

    xt3 = xt_d.ap().rearrange("(c p) n -> c p n", p=P)
    wq3 = wq_d.ap().rearrange("(c p) m -> c p m", p=P)
    wkv3 = wkv_d.ap().rearrange("(c p) m -> c p m", p=P)
    wo3 = wo_d.ap().rearrange("(g p) e -> g p e", p=P)

    with tile.TileContext(nc) as tc, ExitStack() as top:
        pers = top.enter_context(tc.tile_pool(name="pers", bufs=1))
        # qT layout: [d, (qt, g, qi)] -> col = qt*512 + g*128 + qi
        qt_sb = pers.tile([P, G * N], F32R, tag="qt")
        kt_sb = pers.tile([P, N], F32R, tag="kt")          # [d, n]
        v_sb = pers.tile([P, N], F32R, tag="v")            # blk t: v[t*128+p, d]
        cos_sb = pers.tile([P, NT * (D // 2)], F32, tag="cos")
        sin_sb = pers.tile([P, NT * (D // 2)], F32, tag="sin")
        maskt_sb = pers.tile([P, 2 * P], F32, tag="maskt")
        eye_sb = pers.tile([P, P], F32R, tag="eye")
        ones_r = pers.tile([P, 1], F32R, tag="ones_r")     # denominator lhsT


        wo_pool = top.enter_context(tc.tile_pool(name="wo", bufs=G))
        wos = [wo_pool.tile([P, E], F32R, tag="wo", name="wot") for _ in range(G)]

        # ---------------- Phase A: projections + RoPE + transposes ---------
        with ExitStack() as pha:
            wq_pool = pha.enter_context(tc.tile_pool(name="wq", bufs=EC))
            wkv_pool = pha.enter_context(tc.tile_pool(name="wkv", bufs=EC))
            xt_pool = pha.enter_context(tc.tile_pool(name="xtp", bufs=20))
            qrot_pool = pha.enter_context(tc.tile_pool(name="qrot", bufs=2))
            krot_pool = pha.enter_context(tc.tile_pool(name="krot", bufs=2))
            rtmp = pha.enter_context(tc.tile_pool(name="rtmp", bufs=4))
            qps_pool = pha.enter_context(
                tc.tile_pool(name="qps", bufs=2, space="PSUM"))
            kvps_pool = pha.enter_context(
                tc.tile_pool(name="kvps", bufs=2, space="PSUM"))
            trps_pool = pha.enter_context(
                tc.tile_pool(name="trps", bufs=2, space="PSUM"))

            wqs = [wq_pool.tile([P, G * D], F32R, tag="wq", name="wqt")
                   for _ in range(EC)]
            wkvs = [wkv_pool.tile([P, 2 * D], F32R, tag="wkv", name="wkvt")
                    for _ in range(EC)]
            for e in range(EC):
                nc.sync.dma_start(wqs[e][:], wq3[e])

            for qtr in range(4):
                xts = [xt_pool.tile([P, QTR], F32R, tag="xt", name="xtt")
                       for _ in range(EC)]
                for e in range(EC):
                    nc.sync.dma_start(
                        xts[e][:], xt3[e][:, qtr * QTR:(qtr + 1) * QTR])
                if qtr == 0:
                    # everything not needed by the very first matmul group
                    # loads behind the first xT quarter
                    for e in range(EC):
                        nc.sync.dma_start(wkvs[e][:], wkv3[e])
                    nc.sync.dma_start(
                        cos_sb[:].rearrange("p (t d) -> p t d", d=D // 2),
                        cos_d.ap().rearrange("(t p) d -> p t d", p=P))
                    nc.sync.dma_start(
                        sin_sb[:].rearrange("p (t d) -> p t d", d=D // 2),
                        sin_d.ap().rearrange("(t p) d -> p t d", p=P))
                    nc.sync.dma_start(maskt_sb[:], maskt_d.ap())
                    nc.sync.dma_start(eye_sb[:], eye_d.ap())
                    nc.sync.dma_start(ones_r[:], onesr_d.ap())
                if qtr == 1:
                    for g in range(G):
                        nc.sync.dma_start(wos[g][:], wo3[g])

                for t in range(4):
                    T = qtr * 4 + t
                    q_ps = qps_pool.tile([P, G * D], F32, tag="qps")
                    kv_ps = kvps_pool.tile([P, 2 * D], F32, tag="kvps")
                    for e in range(EC):
                        nc.tensor.matmul(
                            q_ps[:], xts[e][:, t * P:(t + 1) * P], wqs[e][:],
                            start=(e == 0), stop=(e == EC - 1))
                    for e in range(EC):
                        nc.tensor.matmul(
                            kv_ps[:], xts[e][:, t * P:(t + 1) * P], wkvs[e][:],
                            start=(e == 0), stop=(e == EC - 1))

                    c_ap = cos_sb[:, T * (D // 2):(T + 1) * (D // 2)]
                    s_ap = sin_sb[:, T * (D // 2):(T + 1) * (D // 2)]
                    q_rot = qrot_pool.tile([P, G * D], F32R, tag="qrot")
                    k_rot = krot_pool.tile([P, D], F32R, tag="krot")
                    _rope(nc, rtmp, q_rot, q_ps, c_ap, s_ap, G)
                    _rope(nc, rtmp, k_rot, kv_ps, c_ap, s_ap, 1)
                    nc.scalar.copy(v_sb[:, T * P:(T + 1) * P], kv_ps[:, D:2 * D])

                    for g in range(G):
                        tq = trps_pool.tile([P, P], F32R, tag="trq", name="trq")
                        nc.tensor.transpose(
                            tq[:], q_rot[:, g * P:(g + 1) * P], eye_sb[:])
                        nc.scalar.copy(
                            qt_sb[:, T * 4 * P + g * P: T * 4 * P + (g + 1) * P],
                            tq[:])
                    tk = trps_pool.tile([P, P], F32R, tag="trq", name="trk")
                    nc.tensor.transpose(tk[:], k_rot[:], eye_sb[:])
                    nc.scalar.copy(kt_sb[:, T * P:(T + 1) * P], tk[:])

        # ---------------- Phase B: attention + output projection -----------
        with ExitStack() as phb:
            ex_pool = phb.enter_context(tc.tile_pool(name="ex", bufs=2))
            smm_pool = phb.enter_context(tc.tile_pool(name="smm", bufs=2))
            stat_pool = phb.enter_context(tc.tile_pool(name="stat", bufs=4))
            bcs_pool = phb.enter_context(tc.tile_pool(name="bcs", bufs=2))
            ao_pool = phb.enter_context(tc.tile_pool(name="aosb", bufs=2))
            osb_pool = phb.enter_context(tc.tile_pool(name="osb", bufs=2))
            sps_pool = phb.enter_context(
                tc.tile_pool(name="sps", bufs=2, space="PSUM"))
            denps_pool = phb.enter_context(
                tc.tile_pool(name="denps", bufs=1, space="PSUM"))
            aops_pool = phb.enter_context(
                tc.tile_pool(name="aops", bufs=2, space="PSUM"))
            wops_pool = phb.enter_context(
                tc.tile_pool(name="wops", bufs=3, space="PSUM"))

            W = G * P  # 512: (g, qi) moving width
            for qt in range(NT):
                nk = min(qt, 2) + 1
                kb0 = qt - (nk - 1)
                exps = ex_pool.tile([P, 3 * W], F32R, tag="exps")
                den_ps = denps_pool.tile([1, W], F32, tag="denps")
                for j in range(nk):
                    kb = kb0 + j
                    dabs = kb - qt          # -2, -1, or 0
                    st_ps = sps_pool.tile([P, W], F32, tag="stps")
                    nc.tensor.matmul(
                        st_ps[:],
                        kt_sb[:, kb * P:(kb + 1) * P],
                        qt_sb[:, qt * W:(qt + 1) * W],
                        start=True, stop=True)
                    eblk = exps[:, j * W:(j + 1) * W]
                    if dabs == -1:
                        nc.scalar.activation(
                            eblk, st_ps[:], mybir.ActivationFunctionType.Exp)
                    else:
                        mblk = maskt_sb[:, 0:P] if dabs == -2 \
                            else maskt_sb[:, P:2 * P]
                        st_sb = smm_pool.tile([P, W], F32, tag="stsb")
                        nc.vector.tensor_add(
                            st_sb[:].rearrange("p (g q) -> p g q", g=G),
                            st_ps[:].rearrange("p (g q) -> p g q", g=G),
                            _bcast_g(mblk))
                        nc.scalar.activation(
                            eblk, st_sb[:], mybir.ActivationFunctionType.Exp)
                    nc.tensor.matmul(
                        den_ps[:], ones_r[:, :1], eblk,
                        start=(j == 0), stop=(j == nk - 1))

                recip = stat_pool.tile([1, W], F32, tag="recip")
                nc.vector.reciprocal_approx_fast(recip[:], den_ps[:])
                bc_sb = bcs_pool.tile([P, W], F32, tag="bcsb")
                nc.gpsimd.partition_broadcast(bc_sb[:], recip[:])

                ao_ps = aops_pool.tile([P, W], F32, tag="aops")
                for j in range(nk):
                    kb = kb0 + j
                    nc.tensor.matmul(
                        ao_ps[:],
                        v_sb[:, kb * P:(kb + 1) * P],
                        exps[:, j * W:(j + 1) * W],
                        start=(j == 0), stop=(j == nk - 1))
                ao_sb = ao_pool.tile([P, W], F32R, tag="aosb")
                nc.vector.tensor_mul(ao_sb[:], ao_ps[:], bc_sb[:])

                out_sb = osb_pool.tile([P, E], F32, tag="outsb")
                for eb in range(4):
                    wo_ps = wops_pool.tile([P, 512], F32, tag="wops")
                    for g in range(G):
                        nc.tensor.matmul(
                            wo_ps[:],
                            ao_sb[:, g * P:(g + 1) * P],
                            wos[g][:, eb * 512:(eb + 1) * 512],
                            start=(g == 0), stop=(g == G - 1))
                    nc.scalar.copy(out_sb[:, eb * 512:(eb + 1) * 512],
                                   wo_ps[:])
                nc.sync.dma_start(out_d.ap()[qt * P:(qt + 1) * P, :], out_sb[:])

    nc.compile()
    return nc


def _host_inputs(x, rope_cos, rope_sin, Wq, Wk, Wv, Wo):
    """Build the 8 per-core input maps."""
    band = np.full((P, 3 * P), MASK_VAL, dtype=np.float32)
    r = np.arange(P)[:, None]
    c = np.arange(3 * P)[None, :]
    band[(c > r) & (c <= r + WIN)] = 0.0
    # transposed mask blocks: [:, :128] for key-tile offset -2,
    # [:, 128:] (causal) for offset 0
    maskt = np.ascontiguousarray(np.concatenate(
        [band[:, 0:P].T, band[:, 2 * P:3 * P].T], axis=1))
    eye = np.eye(P, dtype=np.float32)

    in_maps = []
    for core in range(NCORES):
        b, hk = divmod(core, HK)
        xt = np.ascontiguousarray(x[b].T.astype(np.float32))
        wq = np.ascontiguousarray(
            Wq[:, hk * G * D:(hk + 1) * G * D].astype(np.float32) * SCALE)
        wkv = np.ascontiguousarray(np.concatenate(
            [Wk[:, hk * D:(hk + 1) * D], Wv[:, hk * D:(hk + 1) * D]],
            axis=1).astype(np.float32))
        wo = np.ascontiguousarray(
            Wo[hk * G * D:(hk + 1) * G * D, :].astype(np.float32))
        in_maps.append({
            "xt": xt,
            "wq": wq,
            "wkv": wkv,
            "wo": wo,
            "coss": np.ascontiguousarray(rope_cos[b].astype(np.float32)),
            "sins": np.ascontiguousarray(rope_sin[b].astype(np.float32)),
            "maskt": maskt,
            "eye": eye,
            "onesr": np.ones((P, 1), dtype=np.float32),
        })
    return in_maps


def _run(inputs, trace=False, **kw):
    if "nc" not in _compiled:
        _compiled["nc"] = _build()
    nc = _compiled["nc"]
    in_maps = _host_inputs(**inputs)
    res = run_bass_kernel_spmd(nc, in_maps, list(range(NCORES)), trace=trace, **kw)
    out = np.zeros((B, N, E), dtype=np.float32)
    for core in range(NCORES):
        b = core // HK
        out[b] += res.results[core]["out"]
    return out, res


def kernel(**inputs):
    out, _ = _run(inputs, trace=False)
    return out


# revision 8
# speedup vs baseline: 2.5684x; 1.0079x over previous
"""Grouped-query attention with sliding-window mask on 8 Trainium2 cores.

Sharding: core c handles (batch b = c // 4, kv-head group hk = c % 4).
Each core projects q (4 query heads) / k / v for its group, applies RoPE,
runs windowed attention, and computes a partial output projection
out_partial = attn_heads @ Wo[hk block].  The host sums the 4 partials
per batch.

Attention runs in a scores-transposed layout: S^T[kj, (g,qi)] so all 4
heads share one N=512 moving operand per key tile (the kv head is shared
per group).  Softmax is max-free (scores are bounded ~|5|): the
denominator comes from a ones-vector matmul over exp(S^T), the reciprocal
is partition-broadcast back via a K=1 matmul, and normalization folds
into the PSUM->SBUF move of the attention output.

Matmul operands use float32r (TF32-like, ~1e-4 matmul error, 4x the fp32
rate at moving-dim >= 256).  Problem constants are hardcoded (B=2,
N=2048, E=2048, H=16, G=4, D=128, WIN=256).
"""

import sys

for _p in ("/opt/trn_rl_repo", "/opt/pypackages"):
    if _p not in sys.path:
        sys.path.insert(0, _p)

from contextlib import ExitStack

import numpy as np

import concourse.bacc as bacc
import concourse.bass as bass
import concourse.mybir as mybir
import concourse.tile as tile
from concourse.bass_utils import run_bass_kernel_spmd

B, N, E = 2, 2048, 2048
H, G, WIN = 16, 4, 256
HK = H // G          # 4 kv heads
D = E // H           # 128
SCALE = D ** -0.5
NCORES = 8
P = 128
NT = N // P          # 16 n-tiles
EC = E // P          # 16 contraction chunks
QTR = N // 4         # xT streamed in quarter-columns
F32 = mybir.dt.float32
F32R = mybir.dt.float32r
MASK_VAL = -1.0e30

_compiled = {}


def _rope(nc, rtmp, dst, src, c_ap, s_ap, nblocks):
    """RoPE on [128, nblocks*128] (pairs adjacent along free dim), all
    head-blocks in one strided op set.

    dst[2t]   = src[2t]*cos[t] - src[2t+1]*sin[t]
    dst[2t+1] = src[2t]*sin[t] + src[2t+1]*cos[t]
    """
    sb = src[:]
    db = dst[:]
    hd = D // 2
    x0 = bass.AP(sb.tensor, sb.offset + 0, [sb.ap[0], [P, nblocks], [2, hd]])
    x1 = bass.AP(sb.tensor, sb.offset + 1, [sb.ap[0], [P, nblocks], [2, hd]])
    d0 = bass.AP(db.tensor, db.offset + 0, [db.ap[0], [P, nblocks], [2, hd]])
    d1 = bass.AP(db.tensor, db.offset + 1, [db.ap[0], [P, nblocks], [2, hd]])
    cb = c_ap
    c3 = bass.AP(cb.tensor, cb.offset, [cb.ap[0], [0, nblocks], cb.ap[1]])
    s3 = bass.AP(s_ap.tensor, s_ap.offset, [s_ap.ap[0], [0, nblocks], s_ap.ap[1]])
    t0 = rtmp.tile([P, nblocks * hd], F32, tag="t0", name="t0")
    t1 = rtmp.tile([P, nblocks * hd], F32, tag="t1", name="t1")
    t0v = t0[:].rearrange("p (b d) -> p b d", d=hd)
    t1v = t1[:].rearrange("p (b d) -> p b d", d=hd)
    nc.vector.tensor_mul(t0v, x0, c3)
    nc.vector.tensor_mul(t1v, x1, s3)
    nc.vector.tensor_sub(d0, t0v, t1v)
    t2 = rtmp.tile([P, nblocks * hd], F32, tag="t2", name="t2")
    t3 = rtmp.tile([P, nblocks * hd], F32, tag="t3", name="t3")
    t2v = t2[:].rearrange("p (b d) -> p b d", d=hd)
    t3v = t3[:].rearrange("p (b d) -> p b d", d=hd)
    nc.vector.tensor_mul(t2v, x0, s3)
    nc.vector.tensor_mul(t3v, x1, c3)
    nc.vector.tensor_add(d1, t2v, t3v)


def _bcast_g(ap):
    """[128, 128] AP -> [128, G, 128] with a 0-step head dim."""
    return bass.AP(ap.tensor, ap.offset, [ap.ap[0], [0, G], ap.ap[1]])


def _build():
    nc = bacc.Bacc("TRN2", target_bir_lowering=False, debug=False)

    xt_d = nc.dram_tensor("xt", [E, N], F32R, kind="ExternalInput")
    wq_d = nc.dram_tensor("wq", [E, G * D], F32R, kind="ExternalInput")
    wkv_d = nc.dram_tensor("wkv", [E, 2 * D], F32R, kind="ExternalInput")
    wo_d = nc.dram_tensor("wo", [G * D, E], F32R, kind="ExternalInput")
    cos_d = nc.dram_tensor("coss", [N, D // 2], F32, kind="ExternalInput")
    sin_d = nc.dram_tensor("sins", [N, D // 2], F32, kind="ExternalInput")
    maskt_d = nc.dram_tensor("maskt", [P, 2 * P], F32, kind="ExternalInput")
    eye_d = nc.dram_tensor("eye", [P, P], F32R, kind="ExternalInput")
    onesr_d = nc.dram_tensor("onesr", [P, 1], F32R, kind="ExternalInput")
    out_d = nc.dram_tensor("out", [N, E], F32, kind="ExternalOutput")

    xt3 = xt_d.ap().rearrange("(c p) n -> c p n", p=P)
    wq3 = wq_d.ap().rearrange("(c p) m -> c p m", p=P)
    wkv3 = wkv_d.ap().rearrange("(c p) m -> c p m", p=P)
    wo3 = wo_d.ap().rearrange("(g p) e -> g p e", p=P)

    with tile.TileContext(nc) as tc, ExitStack() as top:
        pers = top.enter_context(tc.tile_pool(name="pers", bufs=1))
        # qT layout: [d, (qt, g, qi)] -> col = qt*512 + g*128 + qi
        qt_sb = pers.tile([P, G * N], F32R, tag="qt")
        kt_sb = pers.tile([P, N], F32R, tag="kt")          # [d, n]
        v_sb = pers.tile([P, N], F32R, tag="v")            # blk t: v[t*128+p, d]
        cos_sb = pers.tile([P, NT * (D // 2)], F32, tag="cos")
        sin_sb = pers.tile([P, NT * (D // 2)], F32, tag="sin")
        maskt_sb = pers.tile([P, 2 * P], F32, tag="maskt")
        eye_sb = pers.tile([P, P], F32R, tag="eye")
        ones_r = pers.tile([P, 1], F32R, tag="ones_r")     # denominator lhsT


        wo_pool = top.enter_context(tc.tile_pool(name="wo", bufs=G))
        wos = [wo_pool.tile([P, E], F32R, tag="wo", name="wot") for _ in range(G)]

        # ---------------- Phase A: projections + RoPE + transposes ---------
        with ExitStack() as pha:
            wq_pool = pha.enter_context(tc.tile_pool(name="wq", bufs=EC))
            wkv_pool = pha.enter_context(tc.tile_pool(name="wkv", bufs=EC))
            xt_pool = pha.enter_context(tc.tile_pool(name="xtp", bufs=20))
            qrot_pool = pha.enter_context(tc.tile_pool(name="qrot", bufs=2))
            krot_pool = pha.enter_context(tc.tile_pool(name="krot", bufs=2))
            rtmp = pha.enter_context(tc.tile_pool(name="rtmp", bufs=4))
            qps_pool = pha.enter_context(
                tc.tile_pool(name="qps", bufs=2, space="PSUM"))
            kvps_pool = pha.enter_context(
                tc.tile_pool(name="kvps", bufs=2, space="PSUM"))
            trps_pool = pha.enter_context(
                tc.tile_pool(name="trps", bufs=2, space="PSUM"))

            wqs = [wq_pool.tile([P, G * D], F32R, tag="wq", name="wqt")
                   for _ in range(EC)]
            wkvs = [wkv_pool.tile([P, 2 * D], F32R, tag="wkv", name="wkvt")
                    for _ in range(EC)]

            for qtr in range(4):
                xts = [xt_pool.tile([P, QTR], F32R, tag="xt", name="xtt")
                       for _ in range(EC)]
                for e in range(EC):
                    if qtr == 0:
                        nc.sync.dma_start(wqs[e][:], wq3[e])
                    nc.sync.dma_start(
                        xts[e][:], xt3[e][:, qtr * QTR:(qtr + 1) * QTR])
                if qtr == 0:
                    # everything not needed by the very first matmul group
                    # loads behind the first xT quarter
                    for e in range(EC):
                        nc.sync.dma_start(wkvs[e][:], wkv3[e])
                    nc.sync.dma_start(
                        cos_sb[:].rearrange("p (t d) -> p t d", d=D // 2),
                        cos_d.ap().rearrange("(t p) d -> p t d", p=P))
                    nc.sync.dma_start(
                        sin_sb[:].rearrange("p (t d) -> p t d", d=D // 2),
                        sin_d.ap().rearrange("(t p) d -> p t d", p=P))
                    nc.sync.dma_start(maskt_sb[:], maskt_d.ap())
                    nc.sync.dma_start(eye_sb[:], eye_d.ap())
                    nc.sync.dma_start(ones_r[:], onesr_d.ap())
                if qtr == 1:
                    for g in range(G):
                        nc.sync.dma_start(wos[g][:], wo3[g])

                for t in range(4):
                    T = qtr * 4 + t
                    q_ps = qps_pool.tile([P, G * D], F32, tag="qps")
                    kv_ps = kvps_pool.tile([P, 2 * D], F32, tag="kvps")
                    for e in range(EC):
                        nc.tensor.matmul(
                            q_ps[:], xts[e][:, t * P:(t + 1) * P], wqs[e][:],
                            start=(e == 0), stop=(e == EC - 1))
                    for e in range(EC):
                        nc.tensor.matmul(
                            kv_ps[:], xts[e][:, t * P:(t + 1) * P], wkvs[e][:],
                            start=(e == 0), stop=(e == EC - 1))

                    c_ap = cos_sb[:, T * (D // 2):(T + 1) * (D // 2)]
                    s_ap = sin_sb[:, T * (D // 2):(T + 1) * (D // 2)]
                    q_rot = qrot_pool.tile([P, G * D], F32R, tag="qrot")
                    k_rot = krot_pool.tile([P, D], F32R, tag="krot")
                    _rope(nc, rtmp, q_rot, q_ps, c_ap, s_ap, G)
                    _rope(nc, rtmp, k_rot, kv_ps, c_ap, s_ap, 1)
                    nc.scalar.copy(v_sb[:, T * P:(T + 1) * P], kv_ps[:, D:2 * D])

                    for g in range(G):
                        tq = trps_pool.tile([P, P], F32R, tag="trq", name="trq")
                        nc.tensor.transpose(
                            tq[:], q_rot[:, g * P:(g + 1) * P], eye_sb[:])
                        nc.scalar.copy(
                            qt_sb[:, T * 4 * P + g * P: T * 4 * P + (g + 1) * P],
                            tq[:])
                    tk = trps_pool.tile([P, P], F32R, tag="trq", name="trk")
                    nc.tensor.transpose(tk[:], k_rot[:], eye_sb[:])
                    nc.scalar.copy(kt_sb[:, T * P:(T + 1) * P], tk[:])

        # ---------------- Phase B: attention + output projection -----------
        with ExitStack() as phb:
            ex_pool = phb.enter_context(tc.tile_pool(name="ex", bufs=2))
            smm_pool = phb.enter_context(tc.tile_pool(name="smm", bufs=2))
            stat_pool = phb.enter_context(tc.tile_pool(name="stat", bufs=4))
            bcs_pool = phb.enter_context(tc.tile_pool(name="bcs", bufs=2))
            ao_pool = phb.enter_context(tc.tile_pool(name="aosb", bufs=2))
            osb_pool = phb.enter_context(tc.tile_pool(name="osb", bufs=2))
            sps_pool = phb.enter_context(
                tc.tile_pool(name="sps", bufs=2, space="PSUM"))
            denps_pool = phb.enter_context(
                tc.tile_pool(name="denps", bufs=1, space="PSUM"))
            aops_pool = phb.enter_context(
                tc.tile_pool(name="aops", bufs=2, space="PSUM"))
            wops_pool = phb.enter_context(
                tc.tile_pool(name="wops", bufs=3, space="PSUM"))

            W = G * P  # 512: (g, qi) moving width
            for qt in range(NT):
                nk = min(qt, 2) + 1
                kb0 = qt - (nk - 1)
                exps = ex_pool.tile([P, 3 * W], F32R, tag="exps")
                den_ps = denps_pool.tile([1, W], F32, tag="denps")
                for j in range(nk):
                    kb = kb0 + j
                    dabs = kb - qt          # -2, -1, or 0
                    st_ps = sps_pool.tile([P, W], F32, tag="stps")
                    nc.tensor.matmul(
                        st_ps[:],
                        kt_sb[:, kb * P:(kb + 1) * P],
                        qt_sb[:, qt * W:(qt + 1) * W],
                        start=True, stop=True)
                    eblk = exps[:, j * W:(j + 1) * W]
                    if dabs == -1:
                        nc.scalar.activation(
                            eblk, st_ps[:], mybir.ActivationFunctionType.Exp)
                    else:
                        mblk = maskt_sb[:, 0:P] if dabs == -2 \
                            else maskt_sb[:, P:2 * P]
                        st_sb = smm_pool.tile([P, W], F32, tag="stsb")
                        nc.vector.tensor_add(
                            st_sb[:].rearrange("p (g q) -> p g q", g=G),
                            st_ps[:].rearrange("p (g q) -> p g q", g=G),
                            _bcast_g(mblk))
                        nc.scalar.activation(
                            eblk, st_sb[:], mybir.ActivationFunctionType.Exp)
                    nc.tensor.matmul(
                        den_ps[:], ones_r[:, :1], eblk,
                        start=(j == 0), stop=(j == nk - 1))

                recip = stat_pool.tile([1, W], F32, tag="recip")
                nc.vector.reciprocal_approx_fast(recip[:], den_ps[:])
                bc_sb = bcs_pool.tile([P, W], F32, tag="bcsb")
                nc.gpsimd.partition_broadcast(bc_sb[:], recip[:])

                ao_ps = aops_pool.tile([P, W], F32, tag="aops")
                for j in range(nk):
                    kb = kb0 + j
                    nc.tensor.matmul(
                        ao_ps[:],
                        v_sb[:, kb * P:(kb + 1) * P],
                        exps[:, j * W:(j + 1) * W],
                        start=(j == 0), stop=(j == nk - 1))
                ao_sb = ao_pool.tile([P, W], F32R, tag="aosb")
                nc.vector.tensor_mul(ao_sb[:], ao_ps[:], bc_sb[:])

                out_sb = osb_pool.tile([P, E], F32, tag="outsb")
                for eb in range(4):
                    wo_ps = wops_pool.tile([P, 512], F32, tag="wops")
                    for g in range(G):
                        nc.tensor.matmul(
                            wo_ps[:],
                            ao_sb[:, g * P:(g + 1) * P],
                            wos[g][:, eb * 512:(eb + 1) * 512],
                            start=(g == 0), stop=(g == G - 1))
                    nc.scalar.copy(out_sb[:, eb * 512:(eb + 1) * 512],
                                   wo_ps[:])
                nc.sync.dma_start(out_d.ap()[qt * P:(qt + 1) * P, :], out_sb[:])

    nc.compile()
    return nc


def _host_inputs(x, rope_cos, rope_sin, Wq, Wk, Wv, Wo):
    """Build the 8 per-core input maps."""
    band = np.full((P, 3 * P), MASK_VAL, dtype=np.float32)
    r = np.arange(P)[:, None]
    c = np.arange(3 * P)[None, :]
    band[(c > r) & (c <= r + WIN)] = 0.0
    # transposed mask blocks: [:, :128] for key-tile offset -2,
    # [:, 128:] (causal) for offset 0
    maskt = np.ascontiguousarray(np.concatenate(
        [band[:, 0:P].T, band[:, 2 * P:3 * P].T], axis=1))
    eye = np.eye(P, dtype=np.float32)

    in_maps = []
    for core in range(NCORES):
        b, hk = divmod(core, HK)
        xt = np.ascontiguousarray(x[b].T.astype(np.float32))
        wq = np.ascontiguousarray(
            Wq[:, hk * G * D:(hk + 1) * G * D].astype(np.float32) * SCALE)
        wkv = np.ascontiguousarray(np.concatenate(
            [Wk[:, hk * D:(hk + 1) * D], Wv[:, hk * D:(hk + 1) * D]],
            axis=1).astype(np.float32))
        wo = np.ascontiguousarray(
            Wo[hk * G * D:(hk + 1) * G * D, :].astype(np.float32))
        in_maps.append({
            "xt": xt,
            "wq": wq,
            "wkv": wkv,
            "wo": wo,
            "coss": np.ascontiguousarray(rope_cos[b].astype(np.float32)),
            "sins": np.ascontiguousarray(rope_sin[b].astype(np.float32)),
            "maskt": maskt,
            "eye": eye,
            "onesr": np.ones((P, 1), dtype=np.float32),
        })
    return in_maps


def _run(inputs, trace=False, **kw):
    if "nc" not in _compiled:
        _compiled["nc"] = _build()
    nc = _compiled["nc"]
    in_maps = _host_inputs(**inputs)
    res = run_bass_kernel_spmd(nc, in_maps, list(range(NCORES)), trace=trace, **kw)
    out = np.zeros((B, N, E), dtype=np.float32)
    for core in range(NCORES):
        b = core // HK
        out[b] += res.results[core]["out"]
    return out, res


def kernel(**inputs):
    out, _ = _run(inputs, trace=False)
    return out
